# revision 11
# baseline (speedup 1.0000x reference)
"""Trainium2 Bass kernel for nn_AlgoGAT (GATv2 GNN, 3 layers, actor-critic heads).

Strategy (8 NeuronCores, SPMD single NEFF):
  - Host: sort edges by dst, partition nodes into 8 contiguous ranges of N/8
    (= 2 graphs per core). Core c owns all in-edges of its nodes, so the
    scatter-softmax is core-local. Edges are grouped into 128-node windows;
    each window's edge list is padded to a multiple of 128 and the per-window
    tile count is equalized across cores (same instruction stream SPMD).
  - Per edge tile [128 edges]: gather xl[src] rows (indirect DMA) from the
    all-gathered table; xe via matmul from transposed edge features; xr via
    one-hot matmul from the window's xr rows; s = xl+xr+xe accumulated in
    PSUM; logit = att . leaky_relu(s); ex = exp(logit) (softmax max-shift
    skipped -- |logit| < 6); scatter (den|xl*ex) into the window accumulator
    with a one-hot matmul.  out = U/den.
  - Between layers, each core computes its slice of next layer's xl table and
    an AllGather distributes it.
  - Heads (pool/action/value/entropy) computed on-device per core (2 graphs).
"""
import math
import numpy as np

import concourse.bacc as bacc
import concourse.bass as bass
import concourse.mybir as mybir
import concourse.tile as tile
from concourse.bass_utils import run_bass_kernel_spmd

F32 = mybir.dt.float32
I32 = mybir.dt.int32
AX = mybir.AxisListType
OP = mybir.AluOpType
ACTF = mybir.ActivationFunctionType

H = 128
HEADS = 2
OC = H // HEADS
SLOPE = 0.2
NF = 64
EF = 32
L = 3
BIG = float(1 << 24)


# ----------------------------------------------------------------------------
# Bass program builder (SPMD; per-core shapes)
# ----------------------------------------------------------------------------

def build_program(ncore, n_nodes, npc, nwin, Tw, epc, graph_nodes, run_layers=L, use_cc=True, do_heads=True):
    """Build the Bass program. All cores run this same program on their own
    data. npc = nodes per core, nwin = windows per core (npc/128),
    Tw[w] = edge tiles in window w (same across cores), epc = 128*sum(Tw),
    graph_nodes = nodes per graph; graphs per core = npc // graph_nodes."""
    ntile = sum(Tw)
    gpc = npc // graph_nodes
    nwin_g = n_nodes // 128          # global windows (all-N prologue)
    cum = np.concatenate([[0], np.cumsum(Tw)]).astype(int)

    nc = bacc.Bacc("TRN2", target_bir_lowering=False, debug=False,
                   num_devices=ncore)

    # ---- external inputs (per core unless noted replicated) ----
    def din(name, shape, dt=F32):
        return nc.dram_tensor(name, list(shape), dt, kind="ExternalInput")

    xT = din("xT", [NF, n_nodes])            # replicated: x^T
    xTloc = din("xTloc", [NF, npc])          # per-core slice of x^T
    eaT = din("eaT", [EF, epc])              # per-core edge_attr^T (padded, sorted)
    dwm = din("dwm", [128, 2 * ntile])       # per-core [dst-in-window | mask] cols
    srcw = din("srcw", [128, ntile], I32)    # per-core src global ids
    Wn = din("Wn", [NF, H])
    bn_c = din("bn_c", [128, 1])
    Wep = din("Wep", [EF, H])
    bep_c = din("bep_c", [128, 1])
    Wl = [din(f"Wl{i}", [H, H]) for i in range(L)]
    bl_r = [din(f"bl_r{i}", [1, H]) for i in range(L)]
    Wr = [din(f"Wr{i}", [H, H]) for i in range(L)]
    br_r = [din(f"br_r{i}", [1, H]) for i in range(L)]
    We = [din(f"We{i}", [H, H]) for i in range(L)]
    att_t = [din(f"att_t{i}", [128, H]) for i in range(L)]
    gbias_t = [din(f"gbias_t{i}", [128, H]) for i in range(L)]
    g_t = [din(f"g_t{i}", [128, H]) for i in range(L)]
    b_t = [din(f"b_t{i}", [128, H]) for i in range(L)]
    gnW = din("gnW", [3 * H, H])
    gnb_r = din("gnb_r", [1, H])
    gng_r = din("gng_r", [1, H])
    gnbb_r = din("gnbb_r", [1, H])
    aW1 = din("aW1", [H, H])
    ab1_c = din("ab1_c", [128, 1])
    aW2_c = din("aW2_c", [H, 1])
    cW1 = din("cW1", [H, H])
    cb1_r = din("cb1_r", [1, H])
    cW2_r = din("cW2_r", [1, H])
    cb2_11 = din("cb2_11", [1, 1])
    ident_in = din("ident", [128, 128])
    iota_c = din("iota_c", [128, 128])       # iota_c[p, j] = j
    iotaMB = din("iotaMB", [1, graph_nodes])  # arange - 2^24
    ones_r = din("ones_r", [1, 128])

    # ---- external outputs ----
    def dout(name, shape, dt=F32):
        return nc.dram_tensor(name, list(shape), dt, kind="ExternalOutput")

    actions_o = dout("actions_o", [1, gpc], I32)
    logp_o = dout("logp_o", [1, gpc])
    ent_o = dout("ent_o", [1, gpc])
    value_o = dout("value_o", [1, gpc])

    # ---- internal DRAM ----
    eT_d = nc.dram_tensor("eT_d", [H, epc], F32)          # relu(ea@Wep+bep)^T
    tab = [nc.dram_tensor(f"tab{i}", [n_nodes, H], F32,
                          addr_space="Shared" if i > 0 else "Local")
           for i in range(L)]
    tloc = nc.dram_tensor("tloc", [npc, H], F32)          # allgather bounce in
    hrow = nc.dram_tensor("hrow", [npc, H], F32)          # residual rows

    with tile.TileContext(nc) as tc:
        import contextlib
        ctx = contextlib.ExitStack()
        with ctx:
            cpool = ctx.enter_context(tc.tile_pool(name="consts", bufs=1))
            work = ctx.enter_context(tc.tile_pool(name="work", bufs=3))
            wwin = ctx.enter_context(tc.tile_pool(name="wwin", bufs=2))
            psA = ctx.enter_context(tc.tile_pool(name="psA", bufs=2, space="PSUM"))
            psB = ctx.enter_context(tc.tile_pool(name="psB", bufs=2, space="PSUM"))
            psO = ctx.enter_context(tc.tile_pool(name="psO", bufs=2, space="PSUM"))
            psW = ctx.enter_context(tc.tile_pool(name="psW", bufs=1, space="PSUM"))

            # ---- load constants to SBUF ----
            _cn = [0]
            def c_load(ap, shape, dt=F32):
                _cn[0] += 1
                t = cpool.tile(list(shape), dt, tag=f"c{_cn[0]}")
                nc.sync.dma_start(out=t[:], in_=ap[:])
                return t

            Wn_s = c_load(Wn, [NF, H])
            bn_s = c_load(bn_c, [128, 1])
            Wep_s = c_load(Wep, [EF, H])
            bep_s = c_load(bep_c, [128, 1])
            Wl_s = [c_load(Wl[i], [H, H]) for i in range(L)]
            bl_s = [c_load(bl_r[i], [1, H]) for i in range(L)]
            Wr_s = [c_load(Wr[i], [H, H]) for i in range(L)]
            br_s = [c_load(br_r[i], [1, H]) for i in range(L)]
            We_s = [c_load(We[i], [H, H]) for i in range(L)]
            att_s = [c_load(att_t[i], [128, H]) for i in range(L)]
            gbias_s = [c_load(gbias_t[i], [128, H]) for i in range(L)]
            g_s = [c_load(g_t[i], [128, H]) for i in range(L)]
            b_s = [c_load(b_t[i], [128, H]) for i in range(L)]
            # gnW lives as [128, 3*H] SBUF tile (partition limit)
            gnW_s = cpool.tile([128, 3 * H], F32, tag="cgnW")
            for j in range(3):
                nc.sync.dma_start(out=gnW_s[:, j * H:(j + 1) * H],
                                  in_=gnW[j * H:(j + 1) * H, :])
            gnb_s = c_load(gnb_r, [1, H])
            gng_s = c_load(gng_r, [1, H])
            gnbb_s = c_load(gnbb_r, [1, H])
            aW1_s = c_load(aW1, [H, H])
            ab1_s = c_load(ab1_c, [128, 1])
            aW2_s = c_load(aW2_c, [H, 1])
            cW1_s = c_load(cW1, [H, H])
            cb1_s = c_load(cb1_r, [1, H])
            cW2_s = c_load(cW2_r, [1, H])
            cb2_s = c_load(cb2_11, [1, 1])
            id_s = c_load(ident_in, [128, 128])
            ioc_s = c_load(iota_c, [128, 128])
            ioMB_s = c_load(iotaMB, [1, graph_nodes])
            ones_s = c_load(ones_r, [1, 128])
            eps_s = cpool.tile([128, 1], F32, tag="ceps")
            nc.vector.memset(eps_s[:], 1e-5)

            # persistent SBUF state
            hT = nc.alloc_sbuf_tensor("hT", [128, npc], F32)       # h^T local
            logits_sb = nc.alloc_sbuf_tensor("logits_sb", [1, npc], F32)

            # =============== PROLOGUE ===============
            # (a) TAB0 = h0@Wl0+bl0 for ALL nodes (replicated on every core)
            for w in range(nwin_g):
                xt = work.tile([NF, 128], F32, tag="xt")
                nc.sync.dma_start(out=xt[:], in_=xT[:, w * 128:(w + 1) * 128])
                ph = psA.tile([128, 128], F32, tag="ps")
                nc.tensor.matmul(out=ph[:], lhsT=Wn_s[:], rhs=xt[:],
                                 start=True, stop=True)
                h0t = work.tile([128, 128], F32, tag="h0t")
                nc.scalar.activation(out=h0t[:], in_=ph[:], func=ACTF.Relu,
                                     bias=bn_s[:])
                pt = psB.tile([128, 128], F32, tag="pT")
                nc.tensor.matmul(out=pt[:], lhsT=h0t[:], rhs=Wl_s[0][:],
                                 start=True, stop=False)
                nc.tensor.matmul(out=pt[:], lhsT=ones_s[:], rhs=bl_s[0][:],
                                 start=False, stop=True)
                t0 = work.tile([128, 128], F32, tag="t0")
                nc.vector.tensor_copy(out=t0[:], in_=pt[:])
                nc.sync.dma_start(out=tab[0][w * 128:(w + 1) * 128, :], in_=t0[:])

            # (b) local h0 -> hT (SBUF) and hrow (DRAM)
            for w in range(nwin):
                xt = work.tile([NF, 128], F32, tag="xt")
                nc.sync.dma_start(out=xt[:], in_=xTloc[:, w * 128:(w + 1) * 128])
                ph = psA.tile([128, 128], F32, tag="ps")
                nc.tensor.matmul(out=ph[:], lhsT=Wn_s[:], rhs=xt[:],
                                 start=True, stop=True)
                nc.scalar.activation(out=hT[:, w * 128:(w + 1) * 128], in_=ph[:],
                                     func=ACTF.Relu, bias=bn_s[:])
                ptr = psB.tile([128, 128], F32, tag="pT")
                nc.tensor.transpose(out=ptr[:], in_=hT[:, w * 128:(w + 1) * 128],
                                    identity=id_s[:])
                hr = work.tile([128, 128], F32, tag="hr")
                nc.vector.tensor_copy(out=hr[:], in_=ptr[:])
                nc.sync.dma_start(out=hrow[w * 128:(w + 1) * 128, :], in_=hr[:])

            # (c) eT = relu(Wep^T @ eaT + bep)
            for g in range(ntile):
                et_in = work.tile([EF, 128], F32, tag="et_in")
                nc.sync.dma_start(out=et_in[:], in_=eaT[:, g * 128:(g + 1) * 128])
                pe = psA.tile([128, 128], F32, tag="ps")
                nc.tensor.matmul(out=pe[:], lhsT=Wep_s[:], rhs=et_in[:],
                                 start=True, stop=True)
                et = work.tile([128, 128], F32, tag="et")
                nc.scalar.activation(out=et[:], in_=pe[:], func=ACTF.Relu,
                                     bias=bep_s[:])
                nc.sync.dma_start(out=eT_d[:, g * 128:(g + 1) * 128], in_=et[:])

            # =============== GAT LAYERS ===============
            for li in range(run_layers):
                for w in range(nwin):
                    # xr rows for this window
                    pxr = psW.tile([128, 128], F32, tag="pxr")
                    nc.tensor.matmul(out=pxr[:], lhsT=hT[:, w * 128:(w + 1) * 128],
                                     rhs=Wr_s[li][:], start=True, stop=False)
                    nc.tensor.matmul(out=pxr[:], lhsT=ones_s[:], rhs=br_s[li][:],
                                     start=False, stop=True)
                    xr = wwin.tile([128, 128], F32, tag="xr")
                    nc.vector.tensor_copy(out=xr[:], in_=pxr[:])
                    # window index/mask/src columns
                    dwm_t = wwin.tile([128, 2 * Tw[w]], F32, tag="dwm")
                    nc.sync.dma_start(out=dwm_t[:],
                                      in_=dwm[:, 2 * cum[w]:2 * cum[w + 1]])
                    src_t = wwin.tile([128, Tw[w]], I32, tag="srt")
                    nc.sync.dma_start(out=src_t[:], in_=srcw[:, cum[w]:cum[w + 1]])
                    out_ps = psO.tile([128, 2 + H], F32, tag="ops")

                    for t in range(Tw[w]):
                        g = cum[w] + t
                        # gather xl rows
                        tabg = work.tile([128, H], F32, tag="tabg")
                        nc.gpsimd.indirect_dma_start(
                            out=tabg[:], out_offset=None,
                            in_=tab[li][:, :],
                            in_offset=bass.IndirectOffsetOnAxis(
                                ap=src_t[:, t:t + 1], axis=0))
                        # edge features (transposed)
                        et = work.tile([128, 128], F32, tag="et")
                        nc.sync.dma_start(out=et[:],
                                          in_=eT_d[:, g * 128:(g + 1) * 128])
                        # one-hot O[e,n]
                        Ot = work.tile([128, 128], F32, tag="Ot")
                        nc.vector.tensor_tensor(
                            out=Ot[:],
                            in0=dwm_t[:, 2 * t:2 * t + 1].to_broadcast([128, 128]),
                            in1=ioc_s[:], op=OP.is_equal)
                        pT = psB.tile([128, 128], F32, tag="pT")
                        nc.tensor.transpose(out=pT[:], in_=Ot[:], identity=id_s[:])
                        OTt = work.tile([128, 128], F32, tag="OTt")
                        nc.vector.tensor_copy(out=OTt[:], in_=pT[:])
                        # s = xe + xr_expand + xl   (PSUM accumulate)
                        ps = psA.tile([128, 128], F32, tag="ps")
                        nc.tensor.matmul(out=ps[:], lhsT=et[:], rhs=We_s[li][:],
                                         start=True, stop=False)
                        nc.tensor.matmul(out=ps[:], lhsT=OTt[:], rhs=xr[:],
                                         start=False, stop=False)
                        nc.tensor.matmul(out=ps[:], lhsT=id_s[:], rhs=tabg[:],
                                         start=False, stop=True)
                        msc = work.tile([128, 128], F32, tag="msc")
                        nc.vector.tensor_scalar(out=msc[:], in0=ps[:],
                                                scalar1=SLOPE, scalar2=None,
                                                op0=OP.mult)
                        m = work.tile([128, 128], F32, tag="m")
                        nc.vector.tensor_tensor(out=m[:], in0=ps[:], in1=msc[:],
                                                op=OP.max)
                        ta = work.tile([128, 128], F32, tag="ta")
                        nc.vector.tensor_tensor(out=ta[:], in0=m[:],
                                                in1=att_s[li][:], op=OP.mult)
                        lg = work.tile([128, HEADS], F32, tag="lg")
                        for hh in range(HEADS):
                            nc.vector.reduce_sum(out=lg[:, hh:hh + 1],
                                                 in_=ta[:, hh * OC:(hh + 1) * OC],
                                                 axis=AX.X)
                        ex = work.tile([128, HEADS], F32, tag="ex")
                        nc.scalar.activation(out=ex[:], in_=lg[:], func=ACTF.Exp)
                        exm = work.tile([128, HEADS], F32, tag="exm")
                        nc.vector.tensor_tensor(
                            out=exm[:], in0=ex[:],
                            in1=dwm_t[:, 2 * t + 1:2 * t + 2].to_broadcast(
                                [128, HEADS]),
                            op=OP.mult)
                        # w = [exm | xl * ex_perhead]
                        wt = work.tile([128, 2 + H], F32, tag="wt")
                        nc.vector.tensor_copy(out=wt[:, 0:2], in_=exm[:])
                        for hh in range(HEADS):
                            nc.vector.tensor_tensor(
                                out=wt[:, 2 + hh * OC:2 + (hh + 1) * OC],
                                in0=tabg[:, hh * OC:(hh + 1) * OC],
                                in1=exm[:, hh:hh + 1].to_broadcast([128, OC]),
                                op=OP.mult)
                        # scatter-accumulate into window accumulator
                        nc.tensor.matmul(out=out_ps[:], lhsT=Ot[:], rhs=wt[:],
                                         start=(t == 0), stop=(t == Tw[w] - 1))

                    # ---- window epilogue: out = U/den, residual, LN ----
                    den = wwin.tile([128, HEADS], F32, tag="den")
                    nc.vector.tensor_scalar(out=den[:], in0=out_ps[:, 0:2],
                                            scalar1=1e-30, scalar2=None,
                                            op0=OP.max)
                    rden = wwin.tile([128, HEADS], F32, tag="rden")
                    nc.vector.reciprocal(out=rden[:], in_=den[:])
                    ow = wwin.tile([128, H], F32, tag="ow")
                    for hh in range(HEADS):
                        nc.vector.tensor_tensor(
                            out=ow[:, hh * OC:(hh + 1) * OC],
                            in0=out_ps[:, 2 + hh * OC:2 + (hh + 1) * OC],
                            in1=rden[:, hh:hh + 1].to_broadcast([128, OC]),
                            op=OP.mult)
                    hold = wwin.tile([128, H], F32, tag="hold")
                    nc.sync.dma_start(out=hold[:],
                                      in_=hrow[w * 128:(w + 1) * 128, :])
                    t1 = wwin.tile([128, H], F32, tag="t1")
                    nc.vector.tensor_tensor(out=t1[:], in0=ow[:],
                                            in1=gbias_s[li][:], op=OP.add)
                    t2 = wwin.tile([128, H], F32, tag="t2")
                    nc.vector.tensor_tensor(out=t2[:], in0=t1[:], in1=hold[:],
                                            op=OP.add)
                    # LayerNorm over features (free dim)
                    mu = wwin.tile([128, 1], F32, tag="mu")
                    nc.vector.reduce_sum(out=mu[:], in_=t2[:], axis=AX.X)
                    nmu = wwin.tile([128, 1], F32, tag="nmu")
                    nc.scalar.activation(out=nmu[:], in_=mu[:], func=ACTF.Copy,
                                         scale=-1.0 / H)
                    xc = wwin.tile([128, H], F32, tag="xc")
                    nc.vector.tensor_scalar(out=xc[:], in0=t2[:], scalar1=nmu[:],
                                            scalar2=None, op0=OP.add)
                    sq = wwin.tile([128, H], F32, tag="sq")
                    nc.vector.tensor_tensor(out=sq[:], in0=xc[:], in1=xc[:],
                                            op=OP.mult)
                    ssq = wwin.tile([128, 1], F32, tag="ssq")
                    nc.vector.reduce_sum(out=ssq[:], in_=sq[:], axis=AX.X)
                    sd = wwin.tile([128, 1], F32, tag="sd")
                    nc.scalar.activation(out=sd[:], in_=ssq[:], func=ACTF.Sqrt,
                                         scale=1.0 / H, bias=eps_s[:])
                    rsd = wwin.tile([128, 1], F32, tag="rsd")
                    nc.vector.reciprocal(out=rsd[:], in_=sd[:])
                    hn = wwin.tile([128, H], F32, tag="hn")
                    nc.vector.tensor_scalar(out=hn[:], in0=xc[:], scalar1=rsd[:],
                                            scalar2=None, op0=OP.mult)
                    hn2 = wwin.tile([128, H], F32, tag="hn2")
                    nc.vector.tensor_tensor(out=hn2[:], in0=hn[:], in1=g_s[li][:],
                                            op=OP.mult)
                    hn3 = wwin.tile([128, H], F32, tag="hn3")
                    nc.vector.tensor_tensor(out=hn3[:], in0=hn2[:], in1=b_s[li][:],
                                            op=OP.add)
                    nc.sync.dma_start(out=hrow[w * 128:(w + 1) * 128, :],
                                      in_=hn3[:])
                    ptr = psB.tile([128, 128], F32, tag="pT")
                    nc.tensor.transpose(out=ptr[:], in_=hn3[:], identity=id_s[:])
                    nc.vector.tensor_copy(out=hT[:, w * 128:(w + 1) * 128],
                                          in_=ptr[:])
                    # next-layer xl slice
                    if li < L - 1:
                        ptab = psW.tile([128, 128], F32, tag="pxr")
                        nc.tensor.matmul(out=ptab[:],
                                         lhsT=hT[:, w * 128:(w + 1) * 128],
                                         rhs=Wl_s[li + 1][:], start=True,
                                         stop=False)
                        nc.tensor.matmul(out=ptab[:], lhsT=ones_s[:],
                                         rhs=bl_s[li + 1][:], start=False,
                                         stop=True)
                        tb = wwin.tile([128, 128], F32, tag="tb")
                        nc.vector.tensor_copy(out=tb[:], in_=ptab[:])
                        nc.sync.dma_start(out=tloc[w * 128:(w + 1) * 128, :],
                                          in_=tb[:])

                if li < L - 1:
                    if use_cc:
                        nc.gpsimd.collective_compute(
                            "AllGather", OP.bypass,
                            replica_groups=[list(range(ncore))],
                            ins=[tloc.ap()], outs=[tab[li + 1].ap()])
                    else:
                        for c2 in range(ncore):
                            nc.sync.dma_start(
                                out=tab[li + 1][c2 * npc:(c2 + 1) * npc, :],
                                in_=tloc[:, :])

            # =============== HEADS ===============
            if not do_heads:
                zf = work.tile([1, gpc], F32, tag="zf")
                zi = work.tile([1, gpc], I32, tag="zi")
                nc.vector.memset(zf[:], 0.0)
                nc.vector.memset(zi[:], 0)
                nc.vector.tensor_copy(out=zf[:, 0:1], in_=hT[0:1, 0:1])
                nc.sync.dma_start(out=actions_o[:, :], in_=zi[:])
                nc.sync.dma_start(out=logp_o[:, :], in_=zf[:])
                nc.sync.dma_start(out=ent_o[:, :], in_=zf[:])
                nc.sync.dma_start(out=value_o[:, :], in_=zf[:])
            # action logits per window
            for w in range(nwin if do_heads else 0):
                pz = psA.tile([128, 128], F32, tag="ps")
                nc.tensor.matmul(out=pz[:], lhsT=aW1_s[:],
                                 rhs=hT[:, w * 128:(w + 1) * 128],
                                 start=True, stop=True)
                zt = work.tile([128, 128], F32, tag="zt")
                nc.scalar.activation(out=zt[:], in_=pz[:], func=ACTF.Relu,
                                     bias=ab1_s[:])
                pl = psB.tile([1, 128], F32, tag="pT")
                nc.tensor.matmul(out=pl[:], lhsT=aW2_s[:], rhs=zt[:],
                                 start=True, stop=True)
                nc.vector.tensor_copy(out=logits_sb[:, w * 128:(w + 1) * 128],
                                      in_=pl[:])

            act_sb = nc.alloc_sbuf_tensor("act_sb", [1, gpc], I32)
            lp_sb = nc.alloc_sbuf_tensor("lp_sb", [1, gpc], F32)
            ent_sb = nc.alloc_sbuf_tensor("ent_sb", [1, gpc], F32)
            val_sb = nc.alloc_sbuf_tensor("val_sb", [1, gpc], F32)

            hpool = ctx.enter_context(tc.tile_pool(name="hpool", bufs=2))
            for gi in range(gpc if do_heads else 0):
                sl = hT[:, gi * graph_nodes:(gi + 1) * graph_nodes]
                # pooled stats (feature-per-partition layout)
                psum_c = hpool.tile([128, 1], F32, tag="psc")
                nc.vector.reduce_sum(out=psum_c[:], in_=sl, axis=AX.X)
                pmean = hpool.tile([128, 1], F32, tag="pmean")
                nc.scalar.activation(out=pmean[:], in_=psum_c[:], func=ACTF.Copy,
                                     scale=1.0 / graph_nodes)
                pmax = hpool.tile([128, 1], F32, tag="pmax")
                nc.vector.reduce_max(out=pmax[:], in_=sl, axis=AX.X)
                pmin = hpool.tile([128, 1], F32, tag="pmin")
                nc.vector.tensor_reduce(out=pmin[:], in_=sl, axis=AX.X,
                                        op=OP.min)
                # gf = ln(relu(concat @ gnW + gnb))
                pgf = psA.tile([1, H], F32, tag="ps")
                nc.tensor.matmul(out=pgf[:], lhsT=pmean[:], rhs=gnW_s[:, 0:H],
                                 start=True, stop=False)
                nc.tensor.matmul(out=pgf[:], lhsT=pmax[:], rhs=gnW_s[:, H:2 * H],
                                 start=False, stop=False)
                nc.tensor.matmul(out=pgf[:], lhsT=pmin[:],
                                 rhs=gnW_s[:, 2 * H:3 * H], start=False,
                                 stop=False)
                nc.tensor.matmul(out=pgf[:], lhsT=ones_s[:, 0:1], rhs=gnb_s[:],
                                 start=False, stop=True)
                gf = hpool.tile([1, H], F32, tag="gf")
                nc.scalar.activation(out=gf[:], in_=pgf[:], func=ACTF.Relu)
                # LN over [1,H]
                gmu = hpool.tile([1, 1], F32, tag="gmu")
                nc.vector.reduce_sum(out=gmu[:], in_=gf[:], axis=AX.X)
                ngmu = hpool.tile([1, 1], F32, tag="ngmu")
                nc.scalar.activation(out=ngmu[:], in_=gmu[:], func=ACTF.Copy,
                                     scale=-1.0 / H)
                gxc = hpool.tile([1, H], F32, tag="gxc")
                nc.vector.tensor_scalar(out=gxc[:], in0=gf[:], scalar1=ngmu[:],
                                        scalar2=None, op0=OP.add)
                gsq = hpool.tile([1, H], F32, tag="gsq")
                nc.vector.tensor_tensor(out=gsq[:], in0=gxc[:], in1=gxc[:],
                                        op=OP.mult)
                gss = hpool.tile([1, 1], F32, tag="gss")
                nc.vector.reduce_sum(out=gss[:], in_=gsq[:], axis=AX.X)
                gsd = hpool.tile([1, 1], F32, tag="gsd")
                nc.scalar.activation(out=gsd[:], in_=gss[:], func=ACTF.Sqrt,
                                     scale=1.0 / H, bias=eps_s[0:1, :])
                grsd = hpool.tile([1, 1], F32, tag="grsd")
                nc.vector.reciprocal(out=grsd[:], in_=gsd[:])
                gn1 = hpool.tile([1, H], F32, tag="gn1")
                nc.vector.tensor_scalar(out=gn1[:], in0=gxc[:], scalar1=grsd[:],
                                        scalar2=None, op0=OP.mult)
                gn2 = hpool.tile([1, H], F32, tag="gn2")
                nc.vector.tensor_tensor(out=gn2[:], in0=gn1[:], in1=gng_s[:],
                                        op=OP.mult)
                gfin = hpool.tile([1, H], F32, tag="gfin")
                nc.vector.tensor_tensor(out=gfin[:], in0=gn2[:], in1=gnbb_s[:],
                                        op=OP.add)
                # value head
                pgt = psB.tile([128, 1], F32, tag="pT")
                nc.tensor.transpose(out=pgt[:], in_=gfin[:],
                                    identity=id_s[0:1, 0:1])
                gcol = hpool.tile([128, 1], F32, tag="gcol")
                nc.vector.tensor_copy(out=gcol[:], in_=pgt[:])
                pv1 = psA.tile([1, H], F32, tag="ps")
                nc.tensor.matmul(out=pv1[:], lhsT=gcol[:], rhs=cW1_s[:],
                                 start=True, stop=False)
                nc.tensor.matmul(out=pv1[:], lhsT=ones_s[:, 0:1], rhs=cb1_s[:],
                                 start=False, stop=True)
                v1 = hpool.tile([1, H], F32, tag="v1")
                nc.scalar.activation(out=v1[:], in_=pv1[:], func=ACTF.Relu)
                vs = hpool.tile([1, H], F32, tag="vs")
                nc.vector.tensor_tensor(out=vs[:], in0=v1[:], in1=cW2_s[:],
                                        op=OP.mult)
                vsum = hpool.tile([1, 1], F32, tag="vsum")
                nc.vector.reduce_sum(out=vsum[:], in_=vs[:], axis=AX.X)
                nc.vector.tensor_tensor(out=val_sb[:, gi:gi + 1], in0=vsum[:],
                                        in1=cb2_s[:], op=OP.add)
                # action distribution on this graph's logits
                row = logits_sb[:, gi * graph_nodes:(gi + 1) * graph_nodes]
                nmx = hpool.tile([1, 1], F32, tag="nmx")
                nc.vector.reduce_max(out=nmx[:], in_=row, axis=AX.X, negate=True)
                exr = hpool.tile([1, graph_nodes], F32, tag="exr")
                nc.scalar.activation(out=exr[:], in_=row, func=ACTF.Exp,
                                     bias=nmx[:])
                S = hpool.tile([1, 1], F32, tag="S")
                nc.vector.reduce_sum(out=S[:], in_=exr[:], axis=AX.X)
                lnS = hpool.tile([1, 1], F32, tag="lnS")
                nc.scalar.activation(out=lnS[:], in_=S[:], func=ACTF.Ln)
                nc.scalar.activation(out=lp_sb[:, gi:gi + 1], in_=lnS[:],
                                     func=ACTF.Copy, scale=-1.0)
                # entropy = lnS - sum(exr*(row-mx))/S
                trow = hpool.tile([1, graph_nodes], F32, tag="trow")
                nc.vector.tensor_scalar(out=trow[:], in0=row, scalar1=nmx[:],
                                        scalar2=None, op0=OP.add)
                scr2 = hpool.tile([1, graph_nodes], F32, tag="scr2")
                nc.vector.tensor_tensor(out=scr2[:], in0=exr[:], in1=trow[:],
                                        op=OP.mult)
                qs = hpool.tile([1, 1], F32, tag="qs")
                nc.vector.reduce_sum(out=qs[:], in_=scr2[:], axis=AX.X)
                rS = hpool.tile([1, 1], F32, tag="rS")
                nc.vector.reciprocal(out=rS[:], in_=S[:])
                qn = hpool.tile([1, 1], F32, tag="qn")
                nc.vector.tensor_tensor(out=qn[:], in0=qs[:], in1=rS[:],
                                        op=OP.mult)
                nc.vector.tensor_tensor(out=ent_sb[:, gi:gi + 1], in0=lnS[:],
                                        in1=qn[:], op=OP.subtract)
                # argmax (first occurrence)
                mx = hpool.tile([1, 1], F32, tag="mx")
                nc.scalar.activation(out=mx[:], in_=nmx[:], func=ACTF.Copy,
                                     scale=-1.0)
                ismax = hpool.tile([1, graph_nodes], F32, tag="ismax")
                nc.vector.tensor_scalar(out=ismax[:], in0=row, scalar1=mx[:],
                                        scalar2=None, op0=OP.is_equal)
                idxv = hpool.tile([1, graph_nodes], F32, tag="idxv")
                nc.vector.tensor_tensor(out=idxv[:], in0=ismax[:], in1=ioMB_s[:],
                                        op=OP.mult)
                idxm = hpool.tile([1, 1], F32, tag="idxm")
                nc.vector.tensor_reduce(out=idxm[:], in_=idxv[:], axis=AX.X,
                                        op=OP.min)
                idxf = hpool.tile([1, 1], F32, tag="idxf")
                nc.scalar.activation(out=idxf[:], in_=idxm[:], func=ACTF.Copy,
                                     bias=BIG)
                nc.vector.tensor_copy(out=act_sb[:, gi:gi + 1], in_=idxf[:])

            if do_heads:
                nc.sync.dma_start(out=actions_o[:, :], in_=act_sb[:])
                nc.sync.dma_start(out=logp_o[:, :], in_=lp_sb[:])
                nc.sync.dma_start(out=ent_o[:, :], in_=ent_sb[:])
                nc.sync.dma_start(out=value_o[:, :], in_=val_sb[:])

    nc.compile()
    return nc


# ----------------------------------------------------------------------------
# Host preprocessing
# ----------------------------------------------------------------------------

def preprocess(x, edge_attr, edge_index, ncore, graph_nodes):
    """Sort/partition/pad edges; build per-core arrays."""
    n_nodes = x.shape[0]
    npc = n_nodes // ncore
    nwin = npc // 128
    src, dst = edge_index[0], edge_index[1]
    perm = np.argsort(dst, kind="stable")
    src_s, dst_s = src[perm], dst[perm]
    ea_s = edge_attr[perm]
    nwin_g = n_nodes // 128
    wcount = np.bincount(dst_s // 128, minlength=nwin_g)
    wtiles = (wcount + 127) // 128
    Tw = wtiles.reshape(ncore, nwin).max(axis=0)         # per-window tiles
    epc = int(Tw.sum() * 128)
    ntile = int(Tw.sum())
    cum = np.concatenate([[0], np.cumsum(Tw)]).astype(int)
    wstart = np.concatenate([[0], np.cumsum(wcount)]).astype(int)

    eaT = np.zeros((ncore, EF, epc), np.float32)
    dwm = np.zeros((ncore, 128, 2 * ntile), np.float32)
    srcw = np.zeros((ncore, 128, ntile), np.int32)
    for c in range(ncore):
        for w in range(nwin):
            gw = c * nwin + w
            s, e = wstart[gw], wstart[gw + 1]
            cnt = e - s
            base = int(cum[w]) * 128
            eaT[c, :, base:base + cnt] = ea_s[s:e].T
            dw = np.full((Tw[w] * 128,), 127.0, np.float32)
            dw[:cnt] = (dst_s[s:e] - gw * 128).astype(np.float32)
            mk = np.zeros((Tw[w] * 128,), np.float32)
            mk[:cnt] = 1.0
            sg = np.zeros((Tw[w] * 128,), np.int32)
            sg[:cnt] = src_s[s:e]
            for t in range(int(Tw[w])):
                dwm[c, :, 2 * (cum[w] + t)] = dw[t * 128:(t + 1) * 128]
                dwm[c, :, 2 * (cum[w] + t) + 1] = mk[t * 128:(t + 1) * 128]
                srcw[c, :, cum[w] + t] = sg[t * 128:(t + 1) * 128]
    return dict(npc=npc, nwin=nwin, Tw=[int(v) for v in Tw], epc=epc,
                eaT=eaT, dwm=dwm, srcw=srcw)


def make_inmaps(inputs, prep, ncore, graph_nodes):
    x = np.ascontiguousarray(inputs["x"], np.float32)
    n_nodes = x.shape[0]
    npc = prep["npc"]
    xT = np.ascontiguousarray(x.T)
    ident = np.eye(128, dtype=np.float32)
    iota_c = np.tile(np.arange(128, dtype=np.float32)[None, :], (128, 1))
    iotaMB = (np.arange(graph_nodes, dtype=np.float32) - BIG)[None, :]
    ones_r = np.ones((1, 128), np.float32)

    def rep(a):
        return np.ascontiguousarray(a.astype(np.float32))

    base = {
        "xT": xT,
        "Wn": rep(inputs["Wn"]), "bn_c": rep(inputs["bn"]).reshape(128, 1),
        "Wep": rep(inputs["Wep"]), "bep_c": rep(inputs["bep"]).reshape(128, 1),
        "gnW": rep(inputs["gnW"]), "gnb_r": rep(inputs["gnb"]).reshape(1, H),
        "gng_r": rep(inputs["gn_g"]).reshape(1, H),
        "gnbb_r": rep(inputs["gn_b"]).reshape(1, H),
        "aW1": rep(inputs["aW1"]), "ab1_c": rep(inputs["ab1"]).reshape(128, 1),
        "aW2_c": rep(inputs["aW2"]).reshape(H, 1),
        "cW1": rep(inputs["cW1"]), "cb1_r": rep(inputs["cb1"]).reshape(1, H),
        "cW2_r": rep(inputs["cW2"]).reshape(1, H),
        "cb2_11": rep(inputs["cb2"]).reshape(1, 1),
        "ident": ident, "iota_c": iota_c, "iotaMB": iotaMB, "ones_r": ones_r,
    }
    for i in range(L):
        base[f"Wl{i}"] = rep(inputs["gWl"][i])
        base[f"bl_r{i}"] = rep(inputs["gbl"][i]).reshape(1, H)
        base[f"Wr{i}"] = rep(inputs["gWr"][i])
        base[f"br_r{i}"] = rep(inputs["gbr"][i]).reshape(1, H)
        base[f"We{i}"] = rep(inputs["gWe"][i])
        base[f"att_t{i}"] = np.tile(
            rep(inputs["gatt"][i]).reshape(1, H), (128, 1))
        base[f"gbias_t{i}"] = np.tile(
            rep(inputs["gbias"][i]).reshape(1, H), (128, 1))
        base[f"g_t{i}"] = np.tile(rep(inputs["ln_g"][i]).reshape(1, H), (128, 1))
        base[f"b_t{i}"] = np.tile(rep(inputs["ln_b"][i]).reshape(1, H), (128, 1))

    in_maps = []
    for c in range(ncore):
        m = dict(base)
        m["xTloc"] = np.ascontiguousarray(xT[:, c * npc:(c + 1) * npc])
        m["eaT"] = np.ascontiguousarray(prep["eaT"][c])
        m["dwm"] = np.ascontiguousarray(prep["dwm"][c])
        m["srcw"] = np.ascontiguousarray(prep["srcw"][c])
        in_maps.append(m)
    return in_maps


_CACHE = {}


def run(inputs, ncore=8, graph_nodes=2048, trace=False):
    x = np.asarray(inputs["x"])
    ea = np.asarray(inputs["edge_attr"], np.float32)
    ei = np.asarray(inputs["edge_index"], np.int32)
    n_nodes = x.shape[0]
    prep = preprocess(np.asarray(x, np.float32), ea, ei, ncore, graph_nodes)
    key = (ncore, n_nodes, tuple(prep["Tw"]), graph_nodes)
    if key not in _CACHE:
        _CACHE[key] = build_program(ncore, n_nodes, prep["npc"], prep["nwin"],
                                    prep["Tw"], prep["epc"], graph_nodes)
    nc = _CACHE[key]
    in_maps = make_inmaps(inputs, prep, ncore, graph_nodes)
    res = run_bass_kernel_spmd(nc, in_maps, core_ids=list(range(ncore)),
                               trace=trace)
    gpc = prep["npc"] // graph_nodes
    B = ncore * gpc
    actions = np.zeros((B,), np.int32)
    logp = np.zeros((B,), np.float32)
    ent = np.zeros((B,), np.float32)
    val = np.zeros((B, 1), np.float32)
    for c in range(ncore):
        r = res.results[c]
        actions[c * gpc:(c + 1) * gpc] = r["actions_o"][0]
        logp[c * gpc:(c + 1) * gpc] = r["logp_o"][0]
        ent[c * gpc:(c + 1) * gpc] = r["ent_o"][0]
        val[c * gpc:(c + 1) * gpc, 0] = r["value_o"][0]
    return (actions, logp, ent, val), res


def kernel(**inputs):
    (actions, logp, ent, val), _ = run(inputs)
    return actions, logp, ent, val
